# revision 5
# baseline (speedup 1.0000x reference)
"""Trainium2 Bass kernel v2: CNModel GNN message passing + common-neighbor scores.

Computes, for N=4096 nodes / E=131072 edges:
    agg  = segment_sum(x[src], dst)          # == A @ x (A dense adjacency)
    h    = relu(agg @ W)                     # W folded into x on host (x@W)
    pred = sigmoid(h.T @ h)

Distribution over 8 NeuronCores (all-static SPMD, one NEFF):
  phase 1: core m computes h rows [512m, 512(m+1)) = (A_T blk).T @ x in two
    2048-column groups; an AllGather per group (fp8) builds full h.
  phase 3: pred is symmetric: only the top half (rows [0,2048)) and the
    bottom-right quadrant are computed; the bottom-left quadrant is
    mirrored on the host from the top-right.  Core m computes pred rows
    [256m, 256(m+1)) over all columns (lhsT = h strip m from half 0) and
    rows [2048+256m, ...) over columns [2048:) (lhsT = h strip 8+m from
    half 1).  Uniform static shape per core; the only rank-dependence is
    one dynamic column offset (256*m) for the two strip loads.
Matmuls run fp8e4 DoubleRow (fp32 PSUM); sigmoid saturates for these
inputs so fp8 quantization is inconsequential.
"""

import numpy as np
import ml_dtypes

N_NODES = 4096
N_CORES = 8
P = 128       # SBUF partitions / PE array dim
GROUP = 2048  # column group per AllGather / rhs tile
STRIP = 256   # pred row-strip per core per half

# matmul unit style: "seq512" = per-psum-tile sequential K chain, 512-wide;
# "seq1024" = same with 1024-wide psum tiles (half the LDWEIGHTS);
# "ws" = weight-stationary interleaved accumulation across 4 psum banks
MM_STYLE = "ws"

_CACHE: dict = {}


def _build_nc(n: int, style: str):
    import concourse.bacc as bacc
    import concourse.bass as bass
    import concourse.mybir as mybir
    import concourse.tile as tile

    dt = mybir.dt
    AFT = mybir.ActivationFunctionType
    DR = mybir.MatmulPerfMode.DoubleRow
    FP8 = dt.float8e4

    blk = n // N_CORES   # 512 h-rows per core
    kt = n // P          # 32 contraction tiles
    half = n // 2

    nc = bacc.Bacc(
        "TRN2", target_bir_lowering=False, debug=False, num_devices=N_CORES
    )
    a_t = nc.dram_tensor("a_t", [n, blk], FP8, kind="ExternalInput").ap()
    x = nc.dram_tensor("x", [n, n], FP8, kind="ExternalInput").ap()
    soff = nc.dram_tensor("soff", [1, 1], dt.uint32, kind="ExternalInput").ap()
    out_full = nc.dram_tensor(
        "out_full", [STRIP, n], dt.float32, kind="ExternalOutput"
    ).ap()
    out_half = nc.dram_tensor(
        "out_half", [STRIP, half], dt.float32, kind="ExternalOutput"
    ).ap()

    ps_bufs = 4 if style == "seq1024" else 8

    with tile.TileContext(nc) as tc:
        with (
            tc.tile_pool(name="dram", bufs=1, space="DRAM") as dram_pool,
            tc.tile_pool(name="lhsT", bufs=1) as lhsT_pool,
            tc.tile_pool(name="rhs", bufs=2) as rhs_pool,
            tc.tile_pool(name="l3", bufs=1) as l3_pool,
            tc.tile_pool(name="ps", bufs=ps_bufs, space="PSUM") as psum_pool,
            tc.tile_pool(name="ev", bufs=4) as ev_pool,
        ):
            h_bounce = [
                dram_pool.tile([blk, GROUP], FP8, name=f"h_bounce{g}")
                for g in range(2)
            ]
            h_half = [
                dram_pool.tile(
                    [n, GROUP], FP8, name=f"h_half{g}", addr_space="Shared"
                )
                for g in range(2)
            ]

            # stationary a_t block in SBUF: [K-part, kt, M=blk]
            at_sb = lhsT_pool.tile([P, kt, blk], FP8, name="at_sb")
            at_src = a_t.rearrange("(kt p) m -> p kt m", p=P)
            for s in range(4):
                ksl = slice(s * (kt // 4), (s + 1) * (kt // 4))
                nc.gpsimd.dma_start(at_sb[:, ksl, :], at_src[:, ksl, :])

            def load_group(src_dram, eng, nsplit=4):
                # src_dram is [n, GROUP]-shaped (or sliced to it)
                t = rhs_pool.tile([P, kt, GROUP], FP8, name="rhs_t", tag="rhs")
                src = src_dram.rearrange("(kt p) f -> p kt f", p=P)
                kstep = kt // nsplit
                for s in range(nsplit):
                    ksl = slice(s * kstep, (s + 1) * kstep)
                    eng.dma_start(t[:, ksl, :], src[:, ksl, :])
                return t

            def mm_unit(lhsT, mt, rhs_t, evict_fn):
                # out rows [mt*P, (mt+1)*P), all GROUP cols, K=n
                if style == "ws":
                    free = 512
                    nsub = GROUP // free
                    pss = [
                        psum_pool.tile([P, free], dt.float32, name="ps",
                                       tag="ps")
                        for _ in range(nsub)
                    ]
                    for k2 in range(kt // 2):
                        for sub in range(nsub):
                            nc.tensor.matmul(
                                pss[sub][:],
                                lhsT[:, 2 * k2:2 * k2 + 2,
                                     mt * P:(mt + 1) * P],
                                rhs_t[:, 2 * k2:2 * k2 + 2,
                                      sub * free:(sub + 1) * free],
                                start=(k2 == 0),
                                stop=(k2 == kt // 2 - 1),
                                perf_mode=DR,
                            )
                    for sub in range(nsub):
                        evict_fn(mt, sub * free, free, pss[sub])
                else:
                    free = 512 if style == "seq512" else 1024
                    for sub in range(GROUP // free):
                        ps = psum_pool.tile([P, free], dt.float32, name="ps",
                                            tag="ps")
                        for k2 in range(kt // 2):
                            nc.tensor.matmul(
                                ps[:],
                                lhsT[:, 2 * k2:2 * k2 + 2,
                                     mt * P:(mt + 1) * P],
                                rhs_t[:, 2 * k2:2 * k2 + 2,
                                      sub * free:(sub + 1) * free],
                                start=(k2 == 0),
                                stop=(k2 == kt // 2 - 1),
                                perf_mode=DR,
                            )
                        evict_fn(mt, sub * free, free, ps)

            def ev_h(g):
                def ev(mt, col, width, ps):
                    hv = ev_pool.tile([P, width], FP8, name="hv", tag="ev8")
                    nc.scalar.activation(hv[:], ps[:], AFT.Relu)
                    nc.scalar.dma_start(
                        h_bounce[g][mt * P:(mt + 1) * P, col:col + width],
                        hv[:],
                    )
                return ev

            def ev_out(dst, coff):
                def ev(mt, col, width, ps):
                    ov = ev_pool.tile([P, width], dt.float32, name="ov",
                                      tag="ev32")
                    nc.scalar.activation(ov[:], ps[:], AFT.Sigmoid)
                    nc.scalar.dma_start(
                        dst[mt * P:(mt + 1) * P, coff + col:coff + col + width],
                        ov[:],
                    )
                return ev

            # ---- phase 1 group 0 + AG1
            rhs_g0 = load_group(x[:, 0:GROUP], nc.sync)
            rhs_g1 = load_group(x[:, GROUP:n], nc.sync)
            for mt in range(blk // P):
                mm_unit(at_sb, mt, rhs_g0, ev_h(0))
            nc.gpsimd.collective_compute(
                "AllGather",
                mybir.AluOpType.bypass,
                replica_groups=[list(range(N_CORES))],
                ins=[h_bounce[0].opt()],
                outs=[h_half[0].opt()],
            )

            # phase-3 strip loads for half 0 (gpsimd: right after AG1)
            regs = nc.alloc_registers("soff_regs")
            nc.regs_load(regs, soff[0:1, 0:1])
            so = nc.snap(regs, donate=True, min_val=0, max_val=GROUP - STRIP)
            kpf = [
                h_half[g].rearrange("(kt p) f -> p kt f", p=P)
                for g in range(2)
            ]
            l3a = l3_pool.tile([P, kt, STRIP], FP8, name="l3a", tag="l3a")
            for s in range(4):
                ksl = slice(s * (kt // 4), (s + 1) * (kt // 4))
                nc.gpsimd.dma_start(
                    l3a[:, ksl, :], kpf[0][:, ksl, bass.ds(so, STRIP)]
                )

            # ---- phase 1 group 1
            for mt in range(blk // P):
                mm_unit(at_sb, mt, rhs_g1, ev_h(1))

            # ---- phase 3 pre-AG2: full strip x gathered half 0
            rhs_h0 = load_group(h_half[0][:, :], nc.sync, nsplit=8)
            for mt in range(STRIP // P):
                mm_unit(l3a, mt, rhs_h0, ev_out(out_full, 0))

            # AG2 (emitted after the pre-work so the gpsimd queue runs
            # l3a loads before blocking on AG2 completion)
            nc.gpsimd.collective_compute(
                "AllGather",
                mybir.AluOpType.bypass,
                replica_groups=[list(range(N_CORES))],
                ins=[h_bounce[1].opt()],
                outs=[h_half[1].opt()],
            )
            l3b = l3_pool.tile([P, kt, STRIP], FP8, name="l3b", tag="l3b")
            for s in range(4):
                ksl = slice(s * (kt // 4), (s + 1) * (kt // 4))
                nc.gpsimd.dma_start(
                    l3b[:, ksl, :], kpf[1][:, ksl, bass.ds(so, STRIP)]
                )

            # ---- phase 3 post-AG2: both strips x gathered half 1
            rhs_h1 = load_group(h_half[1][:, :], nc.sync, nsplit=8)
            for mt in range(STRIP // P):
                mm_unit(l3a, mt, rhs_h1, ev_out(out_full, GROUP))
            for mt in range(STRIP // P):
                mm_unit(l3b, mt, rhs_h1, ev_out(out_half, 0))

    nc.compile()
    return nc


def _get_nc(n: int, style: str = None):
    style = style or MM_STYLE
    key = (n, style)
    if key not in _CACHE:
        _CACHE[key] = _build_nc(n, style)
    return _CACHE[key]


def _kernel_impl(x, edge_index, W, n):
    from concourse.bass_utils import run_bass_kernel_spmd

    fp8 = ml_dtypes.float8_e4m3  # TRN FP8_EXP4: max normal +-240
    x = np.ascontiguousarray(np.asarray(x, dtype=np.float32))
    W = np.asarray(W, dtype=np.float32)
    ei = np.asarray(edge_index)
    src = np.asarray(ei[0], dtype=np.intp)
    dst = np.asarray(ei[1], dtype=np.intp)

    w_is_identity = (
        np.count_nonzero(W) == n and bool((np.diagonal(W) == 1.0).all())
    )
    if not w_is_identity:
        # relu((A@x)@W) == relu(A@(x@W)): fold W into x on the host
        x = np.ascontiguousarray(x @ W)

    # densify edges: A_T[s, d] = multiplicity of edge s->d
    a_t = np.zeros((n, n), dtype=np.float32)
    np.add.at(a_t, (src, dst), 1.0)
    a_t8 = a_t.astype(fp8)
    x8 = np.clip(x, -240.0, 240.0).astype(fp8)

    nc = _get_nc(n)

    blk = n // N_CORES
    half = n // 2
    in_maps = []
    for m in range(N_CORES):
        in_maps.append({
            "a_t": np.ascontiguousarray(a_t8[:, m * blk:(m + 1) * blk]),
            "x": x8,
            "soff": np.array([[STRIP * m]], dtype=np.uint32),
        })

    global LAST_RESULT, LAST_NC, LAST_IN_MAPS
    LAST_NC, LAST_IN_MAPS = nc, in_maps
    res = run_bass_kernel_spmd(nc, in_maps, list(range(N_CORES)))
    LAST_RESULT = res

    out = np.empty((n, n), dtype=np.float32)
    for m in range(N_CORES):
        out[STRIP * m:STRIP * (m + 1), :] = np.asarray(
            res.results[m]["out_full"]
        )
        out[half + STRIP * m:half + STRIP * (m + 1), half:] = np.asarray(
            res.results[m]["out_half"]
        )
    out[half:, :half] = out[:half, half:].T
    return out


LAST_RESULT = None
LAST_NC = None
LAST_IN_MAPS = None


def kernel(x, edge_index, W):
    return _kernel_impl(x, edge_index, W, N_NODES)


# revision 6
# speedup vs baseline: 1.2282x; 1.2282x over previous
"""Trainium2 Bass kernel v2: CNModel GNN message passing + common-neighbor scores.

Computes, for N=4096 nodes / E=131072 edges:
    agg  = segment_sum(x[src], dst)          # == A @ x (A dense adjacency)
    h    = relu(agg @ W)                     # W folded into x on host (x@W)
    pred = sigmoid(h.T @ h)

Distribution over 8 NeuronCores (all-static SPMD, one NEFF):
  phase 1: core m computes h rows [512m, 512(m+1)) = (A_T blk).T @ x in two
    2048-column groups; an AllGather per group (fp8) builds full h.
  phase 3: pred is symmetric: only the top half (rows [0,2048)) and the
    bottom-right quadrant are computed; the bottom-left quadrant is
    mirrored on the host from the top-right.  Core m computes pred rows
    [256m, 256(m+1)) over all columns (lhsT = h strip m from half 0) and
    rows [2048+256m, ...) over columns [2048:) (lhsT = h strip 8+m from
    half 1).  Uniform static shape per core; the only rank-dependence is
    one dynamic column offset (256*m) for the two strip loads.
Matmuls run fp8e4 DoubleRow (fp32 PSUM); sigmoid saturates for these
inputs so fp8 quantization is inconsequential.
"""

import numpy as np
import ml_dtypes

N_NODES = 4096
N_CORES = 8
P = 128       # SBUF partitions / PE array dim
GROUP = 2048  # column group per AllGather / rhs tile
STRIP = 256   # pred row-strip per core per half

# matmul unit style: "seq512" = per-psum-tile sequential K chain, 512-wide;
# "seq1024" = same with 1024-wide psum tiles (half the LDWEIGHTS);
# "ws" = weight-stationary interleaved accumulation across 4 psum banks
MM_STYLE = "ws"

_CACHE: dict = {}


def _build_nc(n: int, style: str):
    import concourse.bacc as bacc
    import concourse.bass as bass
    import concourse.mybir as mybir
    import concourse.tile as tile

    dt = mybir.dt
    AFT = mybir.ActivationFunctionType
    DR = mybir.MatmulPerfMode.DoubleRow
    FP8 = dt.float8e4

    blk = n // N_CORES   # 512 h-rows per core
    kt = n // P          # 32 contraction tiles
    half = n // 2

    nc = bacc.Bacc(
        "TRN2", target_bir_lowering=False, debug=False, num_devices=N_CORES
    )
    a_t = nc.dram_tensor("a_t", [n, blk], FP8, kind="ExternalInput").ap()
    x = nc.dram_tensor("x", [n, n], FP8, kind="ExternalInput").ap()
    soff = nc.dram_tensor("soff", [1, 1], dt.uint32, kind="ExternalInput").ap()
    out_full = nc.dram_tensor(
        "out_full", [STRIP, n], dt.float32, kind="ExternalOutput"
    ).ap()
    out_half = nc.dram_tensor(
        "out_half", [STRIP, half], dt.float32, kind="ExternalOutput"
    ).ap()

    ps_bufs = 4 if style == "seq1024" else 8

    with tile.TileContext(nc) as tc:
        with (
            tc.tile_pool(name="dram", bufs=1, space="DRAM") as dram_pool,
            tc.tile_pool(name="lhsT", bufs=1) as lhsT_pool,
            tc.tile_pool(name="rhs", bufs=2) as rhs_pool,
            tc.tile_pool(name="l3", bufs=1) as l3_pool,
            tc.tile_pool(name="ps", bufs=ps_bufs, space="PSUM") as psum_pool,
            tc.tile_pool(name="ev", bufs=4) as ev_pool,
        ):
            h_bounce = [
                dram_pool.tile([blk, GROUP], FP8, name=f"h_bounce{g}")
                for g in range(2)
            ]
            h_half = [
                dram_pool.tile(
                    [n, GROUP], FP8, name=f"h_half{g}", addr_space="Shared"
                )
                for g in range(2)
            ]

            # stationary a_t block in SBUF: [K-part, kt, M=blk]
            at_sb = lhsT_pool.tile([P, kt, blk], FP8, name="at_sb")
            at_src = a_t.rearrange("(kt p) m -> p kt m", p=P)
            for s in range(4):
                ksl = slice(s * (kt // 4), (s + 1) * (kt // 4))
                nc.gpsimd.dma_start(at_sb[:, ksl, :], at_src[:, ksl, :])

            def load_group(src_dram, eng, nsplit=4):
                # src_dram is [n, GROUP]-shaped (or sliced to it)
                t = rhs_pool.tile([P, kt, GROUP], FP8, name="rhs_t", tag="rhs")
                src = src_dram.rearrange("(kt p) f -> p kt f", p=P)
                kstep = kt // nsplit
                for s in range(nsplit):
                    ksl = slice(s * kstep, (s + 1) * kstep)
                    eng.dma_start(t[:, ksl, :], src[:, ksl, :])
                return t

            def mm_unit(lhsT, mt, rhs_t, evict_fn):
                # out rows [mt*P, (mt+1)*P), all GROUP cols, K=n
                if style == "ws":
                    free = 512
                    nsub = GROUP // free
                    pss = [
                        psum_pool.tile([P, free], dt.float32, name="ps",
                                       tag="ps")
                        for _ in range(nsub)
                    ]
                    for k2 in range(kt // 2):
                        for sub in range(nsub):
                            nc.tensor.matmul(
                                pss[sub][:],
                                lhsT[:, 2 * k2:2 * k2 + 2,
                                     mt * P:(mt + 1) * P],
                                rhs_t[:, 2 * k2:2 * k2 + 2,
                                      sub * free:(sub + 1) * free],
                                start=(k2 == 0),
                                stop=(k2 == kt // 2 - 1),
                                perf_mode=DR,
                            )
                    for sub in range(nsub):
                        evict_fn(mt, sub * free, free, pss[sub])
                else:
                    free = 512 if style == "seq512" else 1024
                    for sub in range(GROUP // free):
                        ps = psum_pool.tile([P, free], dt.float32, name="ps",
                                            tag="ps")
                        for k2 in range(kt // 2):
                            nc.tensor.matmul(
                                ps[:],
                                lhsT[:, 2 * k2:2 * k2 + 2,
                                     mt * P:(mt + 1) * P],
                                rhs_t[:, 2 * k2:2 * k2 + 2,
                                      sub * free:(sub + 1) * free],
                                start=(k2 == 0),
                                stop=(k2 == kt // 2 - 1),
                                perf_mode=DR,
                            )
                        evict_fn(mt, sub * free, free, ps)

            def ev_h(g):
                def ev(mt, col, width, ps):
                    hv = ev_pool.tile([P, width], FP8, name="hv", tag="ev8")
                    nc.scalar.activation(hv[:], ps[:], AFT.Relu)
                    nc.scalar.dma_start(
                        h_bounce[g][mt * P:(mt + 1) * P, col:col + width],
                        hv[:],
                    )
                return ev

            def ev_out(dst, coff):
                def ev(mt, col, width, ps):
                    ov = ev_pool.tile([P, width], dt.float32, name="ov",
                                      tag="ev32")
                    nc.scalar.activation(ov[:], ps[:], AFT.Sigmoid)
                    nc.scalar.dma_start(
                        dst[mt * P:(mt + 1) * P, coff + col:coff + col + width],
                        ov[:],
                    )
                return ev

            # ---- phase 1 group 0 + AG1
            rhs_g0 = load_group(x[:, 0:GROUP], nc.sync)
            rhs_g1 = load_group(x[:, GROUP:n], nc.sync)
            for mt in range(blk // P):
                mm_unit(at_sb, mt, rhs_g0, ev_h(0))
            nc.gpsimd.collective_compute(
                "AllGather",
                mybir.AluOpType.bypass,
                replica_groups=[list(range(N_CORES))],
                ins=[h_bounce[0].opt()],
                outs=[h_half[0].opt()],
            )

            # phase-3 strip loads for half 0 (gpsimd: right after AG1)
            regs = nc.alloc_registers("soff_regs")
            nc.regs_load(regs, soff[0:1, 0:1])
            so = nc.snap(regs, donate=True, min_val=0, max_val=GROUP - STRIP)
            kpf = [
                h_half[g].rearrange("(kt p) f -> p kt f", p=P)
                for g in range(2)
            ]
            l3a = l3_pool.tile([P, kt, STRIP], FP8, name="l3a", tag="l3a")
            for s in range(4):
                ksl = slice(s * (kt // 4), (s + 1) * (kt // 4))
                nc.gpsimd.dma_start(
                    l3a[:, ksl, :], kpf[0][:, ksl, bass.ds(so, STRIP)]
                )

            # ---- phase 1 group 1
            for mt in range(blk // P):
                mm_unit(at_sb, mt, rhs_g1, ev_h(1))

            # ---- phase 3 pre-AG2: full strip x gathered half 0
            rhs_h0 = load_group(h_half[0][:, :], nc.sync)
            for mt in range(STRIP // P):
                mm_unit(l3a, mt, rhs_h0, ev_out(out_full, 0))

            # AG2 (emitted after the pre-work so the gpsimd queue runs
            # l3a loads before blocking on AG2 completion)
            nc.gpsimd.collective_compute(
                "AllGather",
                mybir.AluOpType.bypass,
                replica_groups=[list(range(N_CORES))],
                ins=[h_bounce[1].opt()],
                outs=[h_half[1].opt()],
            )
            l3b = l3_pool.tile([P, kt, STRIP], FP8, name="l3b", tag="l3b")
            for s in range(4):
                ksl = slice(s * (kt // 4), (s + 1) * (kt // 4))
                nc.gpsimd.dma_start(
                    l3b[:, ksl, :], kpf[1][:, ksl, bass.ds(so, STRIP)]
                )

            # ---- phase 3 post-AG2: both strips x gathered half 1
            rhs_h1 = load_group(h_half[1][:, :], nc.sync)
            for mt in range(STRIP // P):
                mm_unit(l3a, mt, rhs_h1, ev_out(out_full, GROUP))
            for mt in range(STRIP // P):
                mm_unit(l3b, mt, rhs_h1, ev_out(out_half, 0))

    nc.compile()
    return nc


def _get_nc(n: int, style: str = None):
    style = style or MM_STYLE
    key = (n, style)
    if key not in _CACHE:
        _CACHE[key] = _build_nc(n, style)
    return _CACHE[key]


def _kernel_impl(x, edge_index, W, n):
    from concourse.bass_utils import run_bass_kernel_spmd

    fp8 = ml_dtypes.float8_e4m3  # TRN FP8_EXP4: max normal +-240
    x = np.ascontiguousarray(np.asarray(x, dtype=np.float32))
    W = np.asarray(W, dtype=np.float32)
    ei = np.asarray(edge_index)
    src = np.asarray(ei[0], dtype=np.intp)
    dst = np.asarray(ei[1], dtype=np.intp)

    w_is_identity = (
        np.count_nonzero(W) == n and bool((np.diagonal(W) == 1.0).all())
    )
    if not w_is_identity:
        # relu((A@x)@W) == relu(A@(x@W)): fold W into x on the host
        x = np.ascontiguousarray(x @ W)

    # densify edges: A_T[s, d] = multiplicity of edge s->d
    a_t = np.zeros((n, n), dtype=np.float32)
    np.add.at(a_t, (src, dst), 1.0)
    a_t8 = a_t.astype(fp8)
    x8 = np.clip(x, -240.0, 240.0).astype(fp8)

    nc = _get_nc(n)

    blk = n // N_CORES
    half = n // 2
    in_maps = []
    for m in range(N_CORES):
        in_maps.append({
            "a_t": np.ascontiguousarray(a_t8[:, m * blk:(m + 1) * blk]),
            "x": x8,
            "soff": np.array([[STRIP * m]], dtype=np.uint32),
        })

    global LAST_RESULT, LAST_NC, LAST_IN_MAPS
    LAST_NC, LAST_IN_MAPS = nc, in_maps
    res = run_bass_kernel_spmd(nc, in_maps, list(range(N_CORES)))
    LAST_RESULT = res

    out = np.empty((n, n), dtype=np.float32)
    for m in range(N_CORES):
        out[STRIP * m:STRIP * (m + 1), :] = np.asarray(
            res.results[m]["out_full"]
        )
        out[half + STRIP * m:half + STRIP * (m + 1), half:] = np.asarray(
            res.results[m]["out_half"]
        )
    out[half:, :half] = out[:half, half:].T
    return out


LAST_RESULT = None
LAST_NC = None
LAST_IN_MAPS = None


def kernel(x, edge_index, W):
    return _kernel_impl(x, edge_index, W, N_NODES)
